# revision 2
# baseline (speedup 1.0000x reference)
"""CTR self-attention kernel for Trainium2 (8 NeuronCores, data-parallel over batch).

Reference computation (per batch b, L=1024, E=O=512, K=4):
    delta = delta_embedding.sum(-1)                       # [L, L]
    valid[i] = i < traj_length[b]
    mask = outer(valid, valid)
    q, k, v = X @ Wq, X @ Wk, X @ Wv                      # [L, O]
    scores = q @ k.T + delta                              # [L, L]
    attn = softmax(scores, axis=-1) * mask                # post-softmax mask
    out = attn @ v                                        # [L, O]

Device mapping (per core: 4 batches):
  - all matmuls on TensorE in float32r (tf32-like, ~11-bit mantissa)
  - delta reduction over K as a one-hot matmul accumulated directly into the
    scores PSUM bank (host pre-transposes delta to [B, L, K, L] so (i,k) is
    the contraction axis and j streams contiguously)
  - softmax: VectorE negated rowmax -> ScalarE Exp(bias=-max) with fused
    row-sum accumulator -> scale folded into the output copy
  - column mask folded into v rows (zero v[j] for j >= t); row mask folded
    into the 1/denominator per-row scale
"""

import numpy as np

B, L, E, O, KD = 32, 1024, 512, 512, 4
NCORES = 8
BPC = B // NCORES  # batches per core

_compiled = None


def _build():
    from contextlib import ExitStack

    import concourse.bass as bass
    import concourse.tile as tile
    from concourse import bacc, mybir

    FP32 = mybir.dt.float32
    FP32R = mybir.dt.float32r
    AX = mybir.AxisListType
    ALU = mybir.AluOpType
    ACTF = mybir.ActivationFunctionType

    nc = bacc.Bacc("TRN2", target_bir_lowering=False, debug=False,
                   num_devices=NCORES)

    NLT = L // 128        # 8 l-tiles (also i-tiles / j-tiles)
    NET = E // 128        # 4 e-tiles
    NOT = O // 128        # 4 o-tiles
    NJC = L // 512        # 2 chunks of 512 along the free dim

    xT_d = nc.dram_tensor("xT", (BPC, E, L), FP32R, kind="ExternalInput")
    dre_d = nc.dram_tensor("dre", (BPC, L * KD, L), FP32R, kind="ExternalInput")
    wq_d = nc.dram_tensor("wq", (E, O), FP32R, kind="ExternalInput")
    wk_d = nc.dram_tensor("wk", (E, O), FP32R, kind="ExternalInput")
    wv_d = nc.dram_tensor("wv", (E, O), FP32R, kind="ExternalInput")
    val_d = nc.dram_tensor("val", (BPC, L), FP32, kind="ExternalInput")
    rg_d = nc.dram_tensor("rg", (NLT // 2, 128, 128), FP32R, kind="ExternalInput")
    id_d = nc.dram_tensor("ident", (128, 128), FP32R, kind="ExternalInput")
    out_d = nc.dram_tensor("out", (BPC, L, O), FP32, kind="ExternalOutput")

    with tile.TileContext(nc) as tc, ExitStack() as ctx:
        cpool = ctx.enter_context(tc.tile_pool(name="const", bufs=1))
        xpool = ctx.enter_context(tc.tile_pool(name="xt", bufs=2))
        qkpool = ctx.enter_context(tc.tile_pool(name="qk", bufs=1))
        vpool = ctx.enter_context(tc.tile_pool(name="v", bufs=1))
        dpool = ctx.enter_context(tc.tile_pool(name="delta", bufs=10))
        ppool = ctx.enter_context(tc.tile_pool(name="p", bufs=2))
        ptpool = ctx.enter_context(tc.tile_pool(name="pt", bufs=2))
        opool = ctx.enter_context(tc.tile_pool(name="osb", bufs=3))
        smpool = ctx.enter_context(tc.tile_pool(name="small", bufs=8))
        vlpool = ctx.enter_context(tc.tile_pool(name="vl", bufs=2))
        # PSUM pools: scores shares its slots with the projection-phase
        # accumulators (tag via same pool), 2 banks per slot.
        scps = ctx.enter_context(tc.tile_pool(name="scps", bufs=2, space="PSUM"))
        trps = ctx.enter_context(tc.tile_pool(name="trps", bufs=2, space="PSUM"))
        ops = ctx.enter_context(tc.tile_pool(name="ops", bufs=2, space="PSUM"))

        # constants
        wq_t = cpool.tile([128, NET, O], FP32R, tag="wq")
        wk_t = cpool.tile([128, NET, O], FP32R, tag="wk")
        wv_t = cpool.tile([128, NET, O], FP32R, tag="wv")
        rg_t = cpool.tile([128, NLT // 2, 128], FP32R, tag="rg")
        id_t = cpool.tile([128, 128], FP32R, tag="ident")
        for t, d in ((wq_t, wq_d), (wk_t, wk_d), (wv_t, wv_d)):
            nc.sync.dma_start(t[:], d[:].rearrange("(et p) o -> p et o", p=128))
        nc.sync.dma_start(rg_t[:], rg_d[:].rearrange("g p m -> p g m"))
        nc.sync.dma_start(id_t[:], id_d[:])

        for b in range(BPC):
            # ---- load xT + valid for this batch ----
            xt = xpool.tile([128, NET, L], FP32R, tag="xt")
            for et in range(NET):
                nc.sync.dma_start(
                    xt[:, et, :],
                    xT_d[b, et * 128:(et + 1) * 128, :],
                )
            vl = vlpool.tile([128, NLT], FP32, tag="vl")
            nc.sync.dma_start(
                vl[:], val_d[b].rearrange("(lt p) -> p lt", p=128)
            )

            # ---- projections ----
            qT = qkpool.tile([128, NOT, L], FP32R, tag="qT")
            kT = qkpool.tile([128, NOT, L], FP32R, tag="kT")
            for (wt, dst) in ((wq_t, qT), (wk_t, kT)):
                for ot in range(NOT):
                    for lc in range(NJC):
                        acc = scps.tile([128, 1024], FP32, tag="sc")
                        for et in range(NET):
                            nc.tensor.matmul(
                                acc[:, 0:512],
                                wt[:, et, ot * 128:(ot + 1) * 128],
                                xt[:, et, lc * 512:(lc + 1) * 512],
                                start=(et == 0), stop=(et == NET - 1),
                            )
                        nc.vector.tensor_copy(
                            dst[:, ot, lc * 512:(lc + 1) * 512], acc[:, 0:512]
                        )

            v_t = vpool.tile([128, NLT, O], FP32R, tag="v")
            for lt in range(NLT):
                acc = scps.tile([128, 1024], FP32, tag="sc")
                for et in range(NET):
                    nc.tensor.matmul(
                        acc[:, 0:512],
                        xt[:, et, lt * 128:(lt + 1) * 128],
                        wv_t[:, et, :],
                        start=(et == 0), stop=(et == NET - 1),
                    )
                # fold column mask: zero rows j >= t
                nc.scalar.activation(
                    v_t[:, lt, :], acc[:, 0:512], ACTF.Copy,
                    bias=0.0, scale=vl[:, lt:lt + 1],
                )

            # ---- attention, one i-tile (128 queries) at a time ----
            for it in range(NLT):
                sc = scps.tile([128, 1024], FP32, tag="sc")
                # scores = q @ k.T
                for jc in range(NJC):
                    for ot in range(NOT):
                        nc.tensor.matmul(
                            sc[:, jc * 512:(jc + 1) * 512],
                            qT[:, ot, it * 128:(it + 1) * 128],
                            kT[:, ot, jc * 512:(jc + 1) * 512],
                            start=(ot == 0), stop=False,
                            skip_group_check=True,
                        )
                # += delta (one-hot reduction over the packed (i,k) axis)
                for g in range(NLT // 2):
                    dl = dpool.tile([128, 1024], FP32R, tag="dl")
                    r0 = (it * 128 + g * 32) * KD
                    nc.sync.dma_start(dl[:], dre_d[b, r0:r0 + 128, :])
                    for jc in range(NJC):
                        nc.tensor.matmul(
                            sc[:, jc * 512:(jc + 1) * 512],
                            rg_t[:, g, :],
                            dl[:, jc * 512:(jc + 1) * 512],
                            start=False, stop=(g == NLT // 2 - 1),
                            skip_group_check=True,
                        )
                # softmax (denominator over the full row, mask applied after)
                nm = smpool.tile([128, 1], FP32, tag="nm")
                nc.vector.tensor_reduce(nm[:], sc[:], axis=AX.X, op=ALU.max,
                                        negate=True)
                p = ppool.tile([128, 1024], FP32R, tag="p")
                den = smpool.tile([128, 1], FP32, tag="den")
                nc.scalar.activation(p[:], sc[:], ACTF.Exp, bias=nm[:],
                                     scale=1.0, accum_out=den[:])
                rs = smpool.tile([128, 1], FP32, tag="rs")
                nc.vector.reciprocal(rs[:], den[:])
                rsm = smpool.tile([128, 1], FP32, tag="rsm")
                # fold row mask into the scale
                nc.vector.tensor_mul(rsm[:], rs[:], vl[:, it:it + 1])

                # transpose p -> pT  (128x128 blocks via TensorE)
                pT = ptpool.tile([128, 1024], FP32R, tag="pT")
                for half in range(2):
                    ptp = trps.tile([128, 512], FP32R, tag="ptp")
                    for tt in range(4):
                        jt = half * 4 + tt
                        nc.tensor.transpose(
                            ptp[:, tt * 128:(tt + 1) * 128],
                            p[:, jt * 128:(jt + 1) * 128],
                            id_t[:],
                        )
                    nc.scalar.copy(
                        pT[:, half * 512:(half + 1) * 512], ptp[:]
                    )

                # out = (p/den * mask) @ v
                op = ops.tile([128, 512], FP32, tag="op")
                for jt in range(NLT):
                    nc.tensor.matmul(
                        op[:],
                        pT[:, jt * 128:(jt + 1) * 128],
                        v_t[:, jt, :],
                        start=(jt == 0), stop=(jt == NLT - 1),
                    )
                ob = opool.tile([128, 512], FP32, tag="ob")
                nc.vector.tensor_scalar(ob[:], op[:], rsm[:], None,
                                        op0=ALU.mult)
                nc.sync.dma_start(
                    out_d[b, it * 128:(it + 1) * 128, :], ob[:]
                )

    nc.compile()
    return nc


def _get_compiled():
    global _compiled
    if _compiled is None:
        _compiled = _build()
    return _compiled


def kernel(joint_embedding, delta_embedding, Wq, Wk, Wv, traj_length):
    from concourse.bass_utils import run_bass_kernel_spmd

    nc = _get_compiled()

    joint_embedding = np.asarray(joint_embedding, dtype=np.float32)
    delta_embedding = np.asarray(delta_embedding, dtype=np.float32)
    Wq = np.asarray(Wq, dtype=np.float32)
    Wk = np.asarray(Wk, dtype=np.float32)
    Wv = np.asarray(Wv, dtype=np.float32)
    traj_length = np.asarray(traj_length)

    valid = (np.arange(L)[None, :] < traj_length[:, None]).astype(np.float32)

    rg = np.zeros((4, 128, 128), dtype=np.float32)
    for ii in range(32):
        for g in range(4):
            rg[g, ii * KD:(ii + 1) * KD, g * 32 + ii] = 1.0
    ident = np.eye(128, dtype=np.float32)

    in_maps = []
    for c in range(NCORES):
        s = c * BPC
        xT = np.ascontiguousarray(
            joint_embedding[s:s + BPC].transpose(0, 2, 1))
        dre = np.ascontiguousarray(
            delta_embedding[s:s + BPC].transpose(0, 1, 3, 2)
        ).reshape(BPC, L * KD, L)
        in_maps.append({
            "xT": xT,
            "dre": dre,
            "wq": Wq, "wk": Wk, "wv": Wv,
            "val": valid[s:s + BPC],
            "rg": rg, "ident": ident,
        })

    res = run_bass_kernel_spmd(nc, in_maps, core_ids=list(range(NCORES)))
    out = np.concatenate([r["out"] for r in res.results], axis=0)
    return out.astype(np.float32)


# revision 3
# speedup vs baseline: 1.0625x; 1.0625x over previous
"""CTR self-attention kernel for Trainium2 (8 NeuronCores, data-parallel over batch).

Reference computation (per batch b, L=1024, E=O=512, K=4):
    delta = delta_embedding.sum(-1)                       # [L, L]
    valid[i] = i < traj_length[b]
    mask = outer(valid, valid)
    q, k, v = X @ Wq, X @ Wk, X @ Wv                      # [L, O]
    scores = q @ k.T + delta                              # [L, L]
    attn = softmax(scores, axis=-1) * mask                # post-softmax mask
    out = attn @ v                                        # [L, O]

Device mapping (per core: 4 batches):
  - all matmuls on TensorE in float32r (tf32-like, ~11-bit mantissa)
  - delta reduction over K as a one-hot matmul accumulated directly into the
    scores PSUM bank (host pre-transposes delta to [B, L, K, L] so (i,k) is
    the contraction axis and j streams contiguously)
  - softmax: VectorE negated rowmax -> ScalarE Exp(bias=-max) with fused
    row-sum accumulator -> scale folded into the output copy
  - column mask folded into v rows (zero v[j] for j >= t); row mask folded
    into the 1/denominator per-row scale
  - DMA load split across both HWDGE rings (SP + ACT)
"""

import os

import numpy as np

B, L, E, O, KD = 32, 1024, 512, 512, 4
NCORES = 8
BPC = B // NCORES  # batches per core

_compiled = None


def _build(reps=1):
    from contextlib import ExitStack

    import concourse.bass as bass
    import concourse.tile as tile
    from concourse import bacc, mybir

    FP32 = mybir.dt.float32
    FP32R = mybir.dt.float32r
    AX = mybir.AxisListType
    ALU = mybir.AluOpType
    ACTF = mybir.ActivationFunctionType

    nc = bacc.Bacc("TRN2", target_bir_lowering=False, debug=False,
                   num_devices=NCORES)

    NLT = L // 128        # 8 l-tiles (also i-tiles / j-tiles)
    NET = E // 128        # 4 e-tiles
    NOT = O // 128        # 4 o-tiles
    NJC = L // 512        # 2 chunks of 512 along the free dim

    xT_d = nc.dram_tensor("xT", (BPC, E, L), FP32R, kind="ExternalInput")
    dre_d = nc.dram_tensor("dre", (BPC, L * KD, L), FP32R, kind="ExternalInput")
    wq_d = nc.dram_tensor("wq", (E, O), FP32R, kind="ExternalInput")
    wk_d = nc.dram_tensor("wk", (E, O), FP32R, kind="ExternalInput")
    wv_d = nc.dram_tensor("wv", (E, O), FP32R, kind="ExternalInput")
    val_d = nc.dram_tensor("val", (BPC, L), FP32, kind="ExternalInput")
    rg_d = nc.dram_tensor("rg", (NLT // 2, 128, 128), FP32R, kind="ExternalInput")
    id_d = nc.dram_tensor("ident", (128, 128), FP32R, kind="ExternalInput")
    out_d = nc.dram_tensor("out", (BPC, L, O), FP32, kind="ExternalOutput")

    with tile.TileContext(nc) as tc, ExitStack() as ctx:
        cpool = ctx.enter_context(tc.tile_pool(name="const", bufs=1))
        xpool = ctx.enter_context(tc.tile_pool(name="xt", bufs=2))
        qkpool = ctx.enter_context(tc.tile_pool(name="qk", bufs=1))
        vpool = ctx.enter_context(tc.tile_pool(name="v", bufs=1))
        dpool = ctx.enter_context(tc.tile_pool(name="delta", bufs=12))
        ppool = ctx.enter_context(tc.tile_pool(name="p", bufs=2))
        ptpool = ctx.enter_context(tc.tile_pool(name="pt", bufs=2))
        opool = ctx.enter_context(tc.tile_pool(name="osb", bufs=3))
        smpool = ctx.enter_context(tc.tile_pool(name="small", bufs=8))
        vlpool = ctx.enter_context(tc.tile_pool(name="vl", bufs=2))
        # PSUM: scores/proj accumulators share slots (2 banks x 2), pT
        # transposes (1 bank x 2), attn@v output (1 bank x 2)
        scps = ctx.enter_context(tc.tile_pool(name="scps", bufs=2, space="PSUM"))
        trps = ctx.enter_context(tc.tile_pool(name="trps", bufs=2, space="PSUM"))
        ops = ctx.enter_context(tc.tile_pool(name="ops", bufs=2, space="PSUM"))

        # DMA ring round-robin: SP and ACT HWDGE rings
        rings = [nc.sync, nc.scalar]
        ring_i = [0]

        def dma(out_ap, in_ap):
            eng = rings[ring_i[0] % 2]
            ring_i[0] += 1
            eng.dma_start(out_ap, in_ap)

        # constants
        wq_t = cpool.tile([128, NET, O], FP32R, tag="wq")
        wk_t = cpool.tile([128, NET, O], FP32R, tag="wk")
        wv_t = cpool.tile([128, NET, O], FP32R, tag="wv")
        rg_t = cpool.tile([128, NLT // 2, 128], FP32R, tag="rg")
        id_t = cpool.tile([128, 128], FP32R, tag="ident")
        for t, d in ((wq_t, wq_d), (wk_t, wk_d), (wv_t, wv_d)):
            dma(t[:], d[:].rearrange("(et p) o -> p et o", p=128))
        dma(rg_t[:], rg_d[:].rearrange("g p m -> p g m"))
        dma(id_t[:], id_d[:])

        for rep in range(reps):
            for b in range(BPC):
                # ---- load xT + valid for this batch ----
                xt = xpool.tile([128, NET, L], FP32R, tag="xt")
                for et in range(NET):
                    dma(xt[:, et, :], xT_d[b, et * 128:(et + 1) * 128, :])
                vl = vlpool.tile([128, NLT], FP32, tag="vl")
                dma(vl[:], val_d[b].rearrange("(lt p) -> p lt", p=128))

                # ---- projections (pairs of accumulators -> PSUM-bank
                # alternation keeps the PE pipelined) ----
                qT = qkpool.tile([128, NOT, L], FP32R, tag="qT")
                kT = qkpool.tile([128, NOT, L], FP32R, tag="kT")
                jobs = [(wq_t, qT, ot, lc) for ot in range(NOT)
                        for lc in range(NJC)]
                jobs += [(wk_t, kT, ot, lc) for ot in range(NOT)
                         for lc in range(NJC)]
                for j0 in range(0, len(jobs), 2):
                    pair = jobs[j0:j0 + 2]
                    accs = []
                    for pi, (wt, dst, ot, lc) in enumerate(pair):
                        acc = scps.tile([128, 1024], FP32, tag="sc",
                                        name=f"acc_{b}_{j0}_{pi}")
                        accs.append(acc)
                    for et in range(NET):
                        for pi, (wt, dst, ot, lc) in enumerate(pair):
                            nc.tensor.matmul(
                                accs[pi][:, 0:512],
                                wt[:, et, ot * 128:(ot + 1) * 128],
                                xt[:, et, lc * 512:(lc + 1) * 512],
                                start=(et == 0), stop=(et == NET - 1),
                            )
                    for pi, (wt, dst, ot, lc) in enumerate(pair):
                        nc.vector.tensor_copy(
                            dst[:, ot, lc * 512:(lc + 1) * 512],
                            accs[pi][:, 0:512],
                        )

                v_t = vpool.tile([128, NLT, O], FP32R, tag="v")
                for lt0 in range(0, NLT, 2):
                    accs = []
                    for pi in range(2):
                        acc = scps.tile([128, 1024], FP32, tag="sc",
                                        name=f"vacc_{b}_{lt0}_{pi}")
                        accs.append(acc)
                    for et in range(NET):
                        for pi in range(2):
                            lt = lt0 + pi
                            nc.tensor.matmul(
                                accs[pi][:, 0:512],
                                xt[:, et, lt * 128:(lt + 1) * 128],
                                wv_t[:, et, :],
                                start=(et == 0), stop=(et == NET - 1),
                            )
                    for pi in range(2):
                        lt = lt0 + pi
                        # fold column mask: zero rows j >= t
                        nc.scalar.activation(
                            v_t[:, lt, :], accs[pi][:, 0:512], ACTF.Copy,
                            bias=0.0, scale=vl[:, lt:lt + 1],
                        )

                # ---- attention, one i-tile (128 queries) at a time ----
                for it in range(NLT):
                    # delta tiles first so their DMAs queue ahead
                    dls = []
                    for g in range(NLT // 2):
                        dl = dpool.tile([128, 1024], FP32R, tag="dl",
                                        name=f"dl_{b}_{it}_{g}")
                        r0 = (it * 128 + g * 32) * KD
                        dma(dl[:], dre_d[b, r0:r0 + 128, :])
                        dls.append(dl)

                    sc = scps.tile([128, 1024], FP32, tag="sc",
                                   name=f"sc_{b}_{it}")
                    # scores = q @ k.T   (jc inner -> bank alternation)
                    for ot in range(NOT):
                        for jc in range(NJC):
                            nc.tensor.matmul(
                                sc[:, jc * 512:(jc + 1) * 512],
                                qT[:, ot, it * 128:(it + 1) * 128],
                                kT[:, ot, jc * 512:(jc + 1) * 512],
                                start=(ot == 0), stop=False,
                                skip_group_check=True,
                            )
                    # += delta (one-hot reduction over the packed (i,k) axis)
                    for g in range(NLT // 2):
                        for jc in range(NJC):
                            nc.tensor.matmul(
                                sc[:, jc * 512:(jc + 1) * 512],
                                rg_t[:, g, :],
                                dls[g][:, jc * 512:(jc + 1) * 512],
                                start=False, stop=(g == NLT // 2 - 1),
                                skip_group_check=True,
                            )
                    # softmax (denominator over the full row; mask after)
                    nm = smpool.tile([128, 1], FP32, tag="nm",
                                     name=f"nm_{b}_{it}")
                    nc.vector.tensor_reduce(nm[:], sc[:], axis=AX.X,
                                            op=ALU.max, negate=True)
                    p = ppool.tile([128, 1024], FP32R, tag="p",
                                   name=f"p_{b}_{it}")
                    den = smpool.tile([128, 1], FP32, tag="den",
                                      name=f"den_{b}_{it}")
                    nc.scalar.activation(p[:], sc[:], ACTF.Exp, bias=nm[:],
                                         scale=1.0, accum_out=den[:])
                    rs = smpool.tile([128, 1], FP32, tag="rs",
                                     name=f"rs_{b}_{it}")
                    nc.vector.reciprocal(rs[:], den[:])
                    rsm = smpool.tile([128, 1], FP32, tag="rsm",
                                      name=f"rsm_{b}_{it}")
                    nc.vector.tensor_mul(rsm[:], rs[:], vl[:, it:it + 1])

                    # transpose p -> pT, alternating the two PSUM slots
                    pT = ptpool.tile([128, 1024], FP32R, tag="pT",
                                     name=f"pT_{b}_{it}")
                    ptps = []
                    for half in range(2):
                        ptp = trps.tile([128, 512], FP32R, tag="ptp",
                                        name=f"ptp_{b}_{it}_{half}")
                        ptps.append(ptp)
                    for tt in range(4):
                        for half in range(2):
                            jt = half * 4 + tt
                            nc.tensor.transpose(
                                ptps[half][:, tt * 128:(tt + 1) * 128],
                                p[:, jt * 128:(jt + 1) * 128],
                                id_t[:],
                            )
                    for half in range(2):
                        nc.scalar.copy(
                            pT[:, half * 512:(half + 1) * 512], ptps[half][:]
                        )

                    # out = (p/den * mask) @ v
                    op = ops.tile([128, 512], FP32, tag="op",
                                  name=f"op_{b}_{it}")
                    for jt in range(NLT):
                        nc.tensor.matmul(
                            op[:],
                            pT[:, jt * 128:(jt + 1) * 128],
                            v_t[:, jt, :],
                            start=(jt == 0), stop=(jt == NLT - 1),
                        )
                    ob = opool.tile([128, 512], FP32, tag="ob",
                                    name=f"ob_{b}_{it}")
                    nc.vector.tensor_scalar(ob[:], op[:], rsm[:], None,
                                            op0=ALU.mult)
                    nc.sync.dma_start(
                        out_d[b, it * 128:(it + 1) * 128, :], ob[:]
                    )

    nc.compile()
    return nc


def _get_compiled():
    global _compiled
    if _compiled is None:
        _compiled = _build(reps=int(os.environ.get("CTR_KERNEL_REPS", "1")))
    return _compiled


def _host_prep(joint_embedding, delta_embedding, Wq, Wk, Wv, traj_length):
    joint_embedding = np.asarray(joint_embedding, dtype=np.float32)
    delta_embedding = np.asarray(delta_embedding, dtype=np.float32)
    valid = (np.arange(L)[None, :] < np.asarray(traj_length)[:, None]
             ).astype(np.float32)

    rg = np.zeros((4, 128, 128), dtype=np.float32)
    for ii in range(32):
        for g in range(4):
            rg[g, ii * KD:(ii + 1) * KD, g * 32 + ii] = 1.0
    ident = np.eye(128, dtype=np.float32)

    in_maps = []
    for c in range(NCORES):
        s = c * BPC
        xT = np.ascontiguousarray(
            joint_embedding[s:s + BPC].transpose(0, 2, 1))
        dre = np.ascontiguousarray(
            delta_embedding[s:s + BPC].transpose(0, 1, 3, 2)
        ).reshape(BPC, L * KD, L)
        in_maps.append({
            "xT": xT,
            "dre": dre,
            "wq": np.asarray(Wq, dtype=np.float32),
            "wk": np.asarray(Wk, dtype=np.float32),
            "wv": np.asarray(Wv, dtype=np.float32),
            "val": valid[s:s + BPC],
            "rg": rg, "ident": ident,
        })
    return in_maps


def kernel(joint_embedding, delta_embedding, Wq, Wk, Wv, traj_length):
    from concourse.bass_utils import run_bass_kernel_spmd

    nc = _get_compiled()
    in_maps = _host_prep(joint_embedding, delta_embedding, Wq, Wk, Wv,
                         traj_length)
    res = run_bass_kernel_spmd(nc, in_maps, core_ids=list(range(NCORES)))
    out = np.concatenate([r["out"] for r in res.results], axis=0)
    return out.astype(np.float32)
